# revision 17
# baseline (speedup 1.0000x reference)
"""GQA decode attention kernel for Trainium2 (8 NeuronCores).

Problem: queries (32,32,1,128) fp32, keys/values (32,8,4096,128) fp32,
GQA group 4 (32 q heads / 8 kv heads), softmax over 4096 keys.

Sharding: batch-parallel. Core i handles batches [4i, 4i+4) -> 32
(batch, kv_head) pairs per core, attention fully local per pair.

Per-pair pipeline (all on-chip tensors bf16 except PSUM accumulators):
  - K streamed HBM->SBUF on the gpsimd (SWDGE) queue with fp32->bf16
    cast during DMA, one full-pair dma_start (16 KiB contiguous per
    partition -> max descriptor size).  kv rows are partition-major
    (partition p holds kv rows p*32..p*32+31); attention is
    permutation-invariant over kv so the permutation is harmless as
    long as K and V share it.
  - V pairs 0..29 streamed fp32 on the sync (HWDGE) queue, full-pair
    dma_starts; cast to bf16 per 8-chunk slice on vector/scalar.
  - V pairs 30..31 land LAST: bf16 cast-during-DMA pieces on the
    gpsimd queue emitted after the final K dma_start.  FIFO order
    makes them the final arrivals, already in matmul dtype, so the
    post-DMA tail is just a few P@V matmuls + the last batch tail.
  - 32 chunks of 128 kv rows per pair:  PE transpose K_c -> K_c^T
    (PSUM), copy to SBUF, scores^T[kv,4] = K_c^T.T @ Q^T via matmul
    into a per-pair PSUM tile [128, 32*4].
  - One fused exp(scale*x) activation (PSUM->SBUF, bf16 out).  Scores
    are ~N(0,1) (max |s| ~ 5.5) so softmax without max-subtraction is
    exact.
  - P@V: out^T[d,4] += V_c.T @ probs^T_c accumulated over chunks in
    PSUM.  Softmax denominators via ones-vector matmul + strided
    free-dim reduces, emitted before the P@V chunk loop so the tail
    never waits on them.
  - Per batch (8 pairs): transpose out^T -> [32,128], scale rows by
    reciprocal sums, store 16 KiB to HBM on the scalar HWDGE queue
    (keeps the sync queue a pure V stream).
"""

import numpy as np

B_PER_CORE = 4      # batches per core
KVH = 8             # kv heads
G = 4               # GQA group size
NH = KVH * G        # query heads
KV = 4096           # kv length
D = 128             # head dim
CH = 32             # kv chunks per pair (KV / 128)
N_CORES = 8
SCALE = 1.0 / float(D) ** 0.5

_CACHE = {}


def _build():
    import concourse.bacc as bacc
    import concourse.mybir as mybir
    from concourse.tile import TileContext
    from concourse.masks import make_identity

    fp32 = mybir.dt.float32
    bf16 = mybir.dt.bfloat16
    AF = mybir.ActivationFunctionType

    nc = bacc.Bacc("TRN2", target_bir_lowering=False)

    q = nc.dram_tensor("q", [B_PER_CORE * NH, D], fp32, kind="ExternalInput")
    k = nc.dram_tensor("k", [B_PER_CORE * KVH, KV, D], fp32, kind="ExternalInput")
    v = nc.dram_tensor("v", [B_PER_CORE * KVH, KV, D], fp32, kind="ExternalInput")
    o = nc.dram_tensor("o", [B_PER_CORE * NH, D], fp32, kind="ExternalOutput")

    NPAIRS = B_PER_CORE * KVH
    N_EARLY = 2   # last two pairs: K loaded + scores computed first
    NLOOP = NPAIRS - N_EARLY
    K_AHEAD = 5   # K loads run this many pairs ahead of the pair loop
    V_AHEAD = 2   # V loads run this many pairs ahead
    VSL = 8       # kv chunks per V cast slice

    with TileContext(nc) as tc:
        with (
            tc.tile_pool(name="const", bufs=1) as const_pool,
            tc.tile_pool(name="kbuf", bufs=K_AHEAD + 4) as k_pool,
            tc.tile_pool(name="vbuf", bufs=V_AHEAD + 3) as v_pool,
            tc.tile_pool(name="vb16", bufs=2 * (CH // VSL)) as vb_pool,
            tc.tile_pool(name="vlh", bufs=3) as vlh_pool,
            tc.tile_pool(name="vlq", bufs=2) as vlq_pool,
            tc.tile_pool(name="kts", bufs=6) as kts_pool,
            tc.tile_pool(name="probs", bufs=6) as probs_pool,
            tc.tile_pool(name="outT", bufs=2) as outTs_pool,
            tc.tile_pool(name="sums", bufs=2) as sums_pool,
            tc.tile_pool(name="small", bufs=2) as small_pool,
            tc.tile_pool(name="outfin", bufs=2) as outfin_pool,
            tc.tile_pool(name="ktp", bufs=4, space="PSUM") as ktp_pool,
            tc.tile_pool(name="stp", bufs=2, space="PSUM") as st_pool,
            tc.tile_pool(name="outTp", bufs=1, space="PSUM") as outTp_pool,
            tc.tile_pool(name="finp", bufs=1, space="PSUM") as fin_pool,
        ):
            kbufs = {}
            vbufs = {}
            vlbufs = {30: [], 31: []}

            def issue_k(p):
                kk = k[p].rearrange("(pp s) d -> pp s d", s=CH)
                t = k_pool.tile([128, CH, D], bf16, tag="kq",
                                name=f"kbuf_{p}")
                nc.gpsimd.dma_start(out=t, in_=kk)
                kbufs[p] = t

            def issue_v(p):
                # fp32 over the HWDGE (sync) queue; full-pair transfer so
                # every descriptor moves 16 KiB contiguous.
                vv = v[p].rearrange("(pp s) d -> pp s d", s=CH)
                t = v_pool.tile([128, CH, D], fp32, tag="vq",
                                name=f"vbuf_{p}")
                nc.sync.dma_start(out=t, in_=vv)
                vbufs[p] = t

            def issue_vl(p, lo, n, pool, tag):
                # last pairs: bf16 cast-during-DMA on the gpsimd queue,
                # emitted after the final K so they arrive last (FIFO).
                vv = v[p].rearrange("(pp s) d -> pp s d", s=CH)
                t = pool.tile([128, n, D], bf16, tag=tag,
                              name=f"vl_{p}_{lo}")
                nc.gpsimd.dma_start(out=t, in_=vv[:, lo:lo + n, :])
                vlbufs[p].append((lo, n, t))

            # V first on the sync queue, then K (early pairs first) on
            # the gpsimd queue; consts follow so no setup delays DMA.
            # The last pairs' V pieces (bf16 cast-during-DMA) are issued
            # UP FRONT and parked in SBUF: the per-engine DMA rings are
            # FIFO, and the engines that also carry the instruction-queue
            # traffic (ACT tables, profiler flushes) run ~20us behind by
            # the end of the stream — nothing the tail depends on may sit
            # at the back of their rings.
            for p in range(V_AHEAD):
                issue_v(p)
            for p in range(NPAIRS - N_EARLY, NPAIRS):
                issue_k(p)
            for p in range(K_AHEAD):
                issue_k(p)

            ident_f = const_pool.tile([128, 128], fp32)
            make_identity(nc, ident_f)
            ident_b = const_pool.tile([128, 128], bf16)
            make_identity(nc, ident_b)
            ones_col = const_pool.tile([128, 1], bf16)
            nc.vector.memset(ones_col, 1.0)

            # Q^T: load all 128 query rows for this core (scalar HWDGE
            # queue), transpose once.
            q_sb = const_pool.tile([128, D], fp32)
            nc.scalar.dma_start(out=q_sb, in_=q[:, :])
            qt_ps = fin_pool.tile([128, 129], fp32, tag="finp")
            nc.tensor.transpose(qt_ps[:, 0:128], q_sb, ident_f)
            qt = const_pool.tile([D, 128], bf16)
            nc.scalar.copy(qt, qt_ps[:, 0:128])

            def scores_phase(p):
                qc = (p // KVH) * NH + (p % KVH) * G
                kb = kbufs.pop(p)
                st_ps = st_pool.tile([128, CH * G], fp32, tag="stp")
                for c in range(CH):
                    ktp = ktp_pool.tile([128, 128], bf16, tag="ktp")
                    nc.tensor.transpose(ktp, kb[:, c, :], ident_b)
                    kts = kts_pool.tile([128, 128], bf16, tag="kts")
                    if c % 3 == 2:
                        nc.scalar.copy(kts, ktp)
                    else:
                        nc.vector.tensor_copy(kts, ktp)
                    nc.tensor.matmul(
                        st_ps[:, c * G:(c + 1) * G],
                        lhsT=kts,
                        rhs=qt[:, qc:qc + G],
                        start=True,
                        stop=True,
                    )
                probs = probs_pool.tile([128, CH * G], bf16, tag="probs")
                nc.scalar.activation(probs, st_ps, AF.Exp, scale=SCALE)
                return probs

            def sums_phase(p, probs, sums_row):
                # softmax denominators; depends only on probs, emitted
                # before the P@V loop so it never gates the tail.
                hk = p % KVH
                sums_ps = fin_pool.tile([1, CH * G], fp32, tag="finp")
                nc.tensor.matmul(sums_ps, lhsT=ones_col, rhs=probs,
                                 start=True, stop=True)
                sv = sums_ps.rearrange("p (c g) -> p c g", g=G)
                for g in range(G):
                    nc.vector.tensor_reduce(
                        sums_row[0:1, hk * G + g:hk * G + g + 1],
                        sv[0:1, :, g],
                        axis=mybir.AxisListType.X,
                        op=mybir.AluOpType.add,
                    )

            def v_phase(p, probs, outT_all, sums_row):
                hk = p % KVH
                sums_phase(p, probs, sums_row)
                vb_full = vbufs.pop(p)
                vb = []
                for si in range(CH // VSL):
                    t = vb_pool.tile([128, VSL, D], bf16, tag="vb")
                    src_sl = vb_full[:, si * VSL:(si + 1) * VSL, :]
                    if si % 2 == 0:
                        nc.vector.tensor_copy(t, src_sl)
                    else:
                        nc.scalar.copy(t, src_sl)
                    vb.append(t)

                outT_ps = outTp_pool.tile([D, G], fp32, tag="outTp")
                for c in range(CH):
                    nc.tensor.matmul(
                        outT_ps,
                        lhsT=vb[c // VSL][:, c % VSL, :],
                        rhs=probs[:, c * G:(c + 1) * G],
                        start=(c == 0),
                        stop=(c == CH - 1),
                    )
                nc.scalar.copy(outT_all[:, hk * G:(hk + 1) * G], outT_ps)

            def v_phase_last(p, probs, outT_all, sums_row):
                # last pairs: V pieces already bf16 (cast during DMA)
                hk = p % KVH
                sums_phase(p, probs, sums_row)
                outT_ps = outTp_pool.tile([D, G], fp32, tag="outTp")
                for lo, n, t in vlbufs[p]:
                    for c in range(lo, lo + n):
                        nc.tensor.matmul(
                            outT_ps,
                            lhsT=t[:, c - lo, :],
                            rhs=probs[:, c * G:(c + 1) * G],
                            start=(c == 0),
                            stop=(c == CH - 1),
                        )
                nc.scalar.copy(outT_all[:, hk * G:(hk + 1) * G], outT_ps)

            def batch_tail(b, outT_all, sums_row):
                # transpose to [rows=32, d=128], scale by 1/sum, store
                fin_ps = fin_pool.tile([128, 129], fp32, tag="finp")
                nc.tensor.transpose(fin_ps[0:NH, 0:128], outT_all, ident_f)
                nc.tensor.transpose(fin_ps[0:NH, 128:129], sums_row,
                                    ident_f[0:1, 0:1])
                recip = small_pool.tile([NH, 1], fp32)
                nc.vector.reciprocal(recip, fin_ps[0:NH, 128:129])
                out_fin = outfin_pool.tile([NH, D], fp32)
                nc.scalar.activation(out_fin, fin_ps[0:NH, 0:128], AF.Copy,
                                     scale=recip)
                nc.scalar.dma_start(out=o[b * NH:(b + 1) * NH, :], in_=out_fin)

            probs_late = {}
            for p in range(NPAIRS - N_EARLY, NPAIRS):
                probs_late[p] = scores_phase(p)

            batch_state = {}
            for p in range(NLOOP):
                b, hk = divmod(p, KVH)
                if hk == 0:
                    batch_state[b] = (
                        outTs_pool.tile([D, NH], fp32, tag="outT",
                                        name=f"outT_all_{b}"),
                        sums_pool.tile([1, NH], fp32, tag="sums",
                                       name=f"sums_row_{b}"),
                    )
                if p + K_AHEAD < NLOOP:
                    issue_k(p + K_AHEAD)
                # the last pairs' V pieces go on the gpsimd queue right
                # after the final K dma_start
                elif p == NLOOP - K_AHEAD:
                    issue_vl(30, 0, 16, vlh_pool, "vlh")
                    issue_vl(30, 16, 16, vlh_pool, "vlh")
                elif p == NLOOP - K_AHEAD + 1:
                    issue_vl(31, 0, 16, vlh_pool, "vlh")
                elif p == NLOOP - K_AHEAD + 2:
                    issue_vl(31, 16, 8, vlq_pool, "vlq")
                    issue_vl(31, 24, 8, vlq_pool, "vlq")
                if p + V_AHEAD < NLOOP:
                    issue_v(p + V_AHEAD)
                probs = scores_phase(p)
                v_phase(p, probs, *batch_state[b])
                if hk == KVH - 1:
                    batch_tail(b, *batch_state[b])

            for p in range(NPAIRS - N_EARLY, NPAIRS):
                v_phase_last(p, probs_late[p], *batch_state[B_PER_CORE - 1])
            batch_tail(B_PER_CORE - 1, *batch_state[B_PER_CORE - 1])

    nc.compile()
    return nc


_TRACE = False
_LAST_RESULTS = None


def kernel(queries, keys, values, mask=None, **_ignored):
    global _LAST_RESULTS
    from concourse.bass_utils import run_bass_kernel_spmd

    if "nc" not in _CACHE:
        _CACHE["nc"] = _build()
    nc = _CACHE["nc"]

    queries = np.ascontiguousarray(np.asarray(queries, dtype=np.float32))
    keys = np.ascontiguousarray(np.asarray(keys, dtype=np.float32))
    values = np.ascontiguousarray(np.asarray(values, dtype=np.float32))

    in_maps = []
    for i in range(N_CORES):
        b0 = i * B_PER_CORE
        b1 = b0 + B_PER_CORE
        in_maps.append({
            "q": np.ascontiguousarray(
                queries[b0:b1].reshape(B_PER_CORE * NH, D)),
            "k": np.ascontiguousarray(
                keys[b0:b1].reshape(B_PER_CORE * KVH, KV, D)),
            "v": np.ascontiguousarray(
                values[b0:b1].reshape(B_PER_CORE * KVH, KV, D)),
        })

    res = run_bass_kernel_spmd(
        nc, in_maps, core_ids=list(range(N_CORES)), trace=_TRACE,
    )
    _LAST_RESULTS = res

    out = np.concatenate(
        [r["o"].reshape(B_PER_CORE, NH, 1, D) for r in res.results], axis=0
    )
    return out
